# revision 29
# baseline (speedup 1.0000x reference)
"""Trainium2 Bass kernel for the recurrent STP network (nn_Network_20109036880204).

Strategy: tensor-parallel over the output-neuron dim across 8 NeuronCores.
  - Each core owns a 1024-neuron shard: W_c = Wab[c*1024:(c+1)*1024, :]^T,
    stored fp16 resident in SBUF as 64 K-tiles [128, 1024] (128 KiB/partition).
  - All [B, N] state tensors live in SBUF in "state layout": tile [128, 256]
    with  tile[p, j*32 + b] = state[b, n = c*1024 + j*128 + p].
    (n on partitions -> fast 128-lane elementwise AND the matmul's stationary
    operand y^T [128, 32] is a contiguous free-dim slice.)
  - Per step: y = u'*x'*r (fp16) -> DRAM -> AllGather(8) -> y_full in SBUF ->
    128 matmuls (K=8192 in 64 tiles, N=1024 in 2 PSUM chunks) -> PE transpose
    of the [32, 1024] result back into state layout -> fused DVE update chain.
"""

import sys

for _p in ("/opt/trn_rl_repo", "/root/.axon_site/_ro/trn_rl_repo"):
    if _p not in sys.path:
        sys.path.append(_p)

import numpy as np

import concourse.bass as bass
import concourse.bacc as bacc
import concourse.mybir as mybir
import concourse.tile as tile
from concourse import bass_utils, masks

# problem constants
NCORES = 8
B = 32
N = 8192
NS = N // NCORES          # 1024 neurons per core
P = 128
J = NS // P               # 8 local K-tiles per core
T = N // P                # 64 K-tiles total
F = J * B                 # 256 = free size of a state tile
CHUNK = 512               # matmul moving free dim (one PSUM bank)
NCH = NS // CHUNK         # 2 chunks

DT = 0.01
USE = 0.03
TAU_FAC = 1.0
TAU_REC = 0.25
C1 = DT / TAU_FAC         # 0.01
C0 = DT * USE / TAU_FAC   # 3e-4
A1 = USE * DT             # 3e-4
C2 = DT / TAU_REC         # 0.04

F32 = mybir.dt.float32
F16 = mybir.dt.float16
MULT = mybir.AluOpType.mult
ADD = mybir.AluOpType.add
MAX = mybir.AluOpType.max


# Skewed A/B split: half A = first JA j-blocks (gathered early, small so its
# AllGather completes by matmul-end), half B = the rest.
JA = 3
JB = J - JA
HA = JA * B               # 96  = state-free width of half A
HB = JB * B               # 160 = width of half B
HW = {"A": HA, "B": HB}
A_TILES = [t for t in range(T) if t % J < JA]
B_TILES = [t for t in range(T) if t % J >= JA]
# output column groups (psum free widths; each <= 512 = one bank)
G_BOUNDS = [0, JA * P, JA * P + 4 * P, NS]      # [0, 384, 896, 1024]
NG = len(G_BOUNDS) - 1
# j-block -> group index
J_GROUP = [next(g for g in range(NG)
                if G_BOUNDS[g] <= j * P < G_BOUNDS[g + 1]) for j in range(J)]


def build_program(n_steps: int, uni=(None, None, None, None), n_dummy=8):
    """Build the SPMD Bass program (identical on all 8 cores).

    Two-half pipeline: each core's y shard is split into half A (j=0..3)
    and half B (j=4..7); each half is all-gathered separately so AG_A can
    fly while the tail of the matmul still runs, and the next step's
    matmul consumes A-sourced K-tiles first.
    """
    es_v, ds_v, e_v, dt_v = uni  # uniform values of the const vectors, or None

    nc = bacc.Bacc(
        "TRN2",
        target_bir_lowering=False,
        debug=False,
        num_devices=NCORES,
    )

    w_dram = nc.dram_tensor("w", [T, P, NS], F16, kind="ExternalInput")
    sd = {
        nm: nc.dram_tensor(nm, [P, F], F32, kind="ExternalInput")
        for nm in ["r0", "recs0", "u0", "x0", "ff", "es", "ds", "e", "dt"]
    }
    r_out = nc.dram_tensor("r_out", [P, F], F32, kind="ExternalOutput")

    with tile.TileContext(nc) as tc:
        with (
            tc.tile_pool(name="wpool", bufs=1) as wpool,
            tc.tile_pool(name="cpool", bufs=1) as cpool,
            tc.tile_pool(name="spool", bufs=2) as spool,
            tc.tile_pool(name="wk", bufs=2) as wk,
            tc.tile_pool(name="yp", bufs=2) as yp,
            tc.tile_pool(name="pmm", bufs=2, space="PSUM") as pmm,
            tc.tile_pool(name="pT", bufs=2, space="PSUM") as pT,
            tc.tile_pool(name="dp", bufs=3, space="DRAM") as dp,
        ):
            # ---- resident weights: 16 DMAs so they spread across queues ----
            w_sb = wpool.tile([P, T * NS], F16, tag="w")
            TB = 4  # K-tiles per DMA
            for i in range(T // TB):
                dst = w_sb[:, i * TB * NS:(i + 1) * TB * NS].rearrange(
                    "p (t n) -> p t n", t=TB
                )
                src = w_dram[i * TB:(i + 1) * TB, :, :].rearrange("t p n -> p t n")
                nc.sync.dma_start(dst, src)

            # ---- constants / initial state ----
            ff_sb = cpool.tile([P, F], F32, tag="ff")
            es_sb = cpool.tile([P, F], F32, tag="es")
            ds_sb = cpool.tile([P, F], F32, tag="ds")
            e_sb = cpool.tile([P, F], F32, tag="e")
            dt_sb = cpool.tile([P, F], F32, tag="dt")
            ident = cpool.tile([B, B], F32, tag="ident")
            for t_, nm in [(ff_sb, "ff"), (es_sb, "es"), (ds_sb, "ds"),
                           (e_sb, "e"), (dt_sb, "dt")]:
                nc.sync.dma_start(t_[:], sd[nm][:])
            masks.make_identity(nc, ident[:])

            r = spool.tile([P, F], F32, tag="r")
            recS = spool.tile([P, F], F32, tag="recS")
            u0_sb = wk.tile([P, F], F32, tag="u0", bufs=1)
            x0_sb = wk.tile([P, F], F32, tag="x0", bufs=1)
            for t_, nm in [(r, "r0"), (recS, "recs0"), (u0_sb, "u0"),
                           (x0_sb, "x0")]:
                nc.sync.dma_start(t_[:], sd[nm][:])

            V = nc.vector

            # ---- prologue: u1, x1, y0 from initial state ----
            s1 = wk.tile([P, F], F32, tag="t0", bufs=1)
            m = wk.tile([P, F], F32, tag="t1", bufs=1)
            s2 = wk.tile([P, F], F32, tag="t2", bufs=1)
            un = spool.tile([P, F], F32, tag="u")
            V.tensor_scalar(s1[:], u0_sb[:], 1.0 - C1, C0, MULT, ADD)
            V.tensor_mul(m[:], u0_sb[:], r[:])
            V.scalar_tensor_tensor(s2[:], r[:], A1, s1[:], MULT, ADD)
            V.scalar_tensor_tensor(un[:], m[:], -A1, s2[:], MULT, ADD)

            t2p = wk.tile([P, F], F32, tag="t3", bufs=1)
            t3p = wk.tile([P, F], F32, tag="t4", bufs=1)
            s4 = wk.tile([P, F], F32, tag="t5", bufs=1)
            xn = spool.tile([P, F], F32, tag="x")
            V.tensor_mul(t2p[:], x0_sb[:], r[:])
            V.tensor_mul(t3p[:], un[:], t2p[:])
            V.tensor_scalar(s4[:], x0_sb[:], 1.0 - C2, C2, MULT, ADD)
            V.scalar_tensor_tensor(xn[:], t3p[:], -DT, s4[:], MULT, ADD)

            w0 = wk.tile([P, F], F32, tag="t6", bufs=1)
            yh = {}
            V.tensor_mul(w0[:], un[:], xn[:])
            for hf, sl in (("A", slice(0, HA)), ("B", slice(HA, F))):
                yh[hf] = yp.tile([P, HW[hf]], F16, tag=f"y{hf}",
                                 name=f"y{hf}_pro")
                V.tensor_mul(yh[hf][:], w0[:, sl], r[:, sl])

            ag_counter = [0]

            def launch_ag(hf, ytile):
                """store y-half to DRAM, AllGather, DMA gathered tiles back."""
                k = ag_counter[0] = ag_counter[0] + 1
                w_ = HW[hf]
                ydr = dp.tile([P, w_], F16, tag=f"ydr{hf}", name=f"ydr{hf}_{k}")
                # store issued from gpsimd so the collective doorbell follows
                # it on the same queue (no cross-engine semaphore hop)
                nc.gpsimd.dma_start(ydr[:], ytile[:])
                yall = dp.tile([NCORES, P, w_], F16, tag=f"yall{hf}",
                               name=f"yall{hf}_{k}")
                nc.gpsimd.collective_compute(
                    "AllGather",
                    mybir.AluOpType.bypass,
                    replica_groups=[list(range(NCORES))],
                    ins=[ydr.opt()],
                    outs=[yall.opt()],
                )
                yfull = yp.tile([P, NCORES * w_], F16, tag=f"yfull{hf}",
                                name=f"yfull{hf}_{k}")
                # block c=0 first (tiny DMA) so the next step's first
                # matmuls ungate as early as possible
                nc.sync.dma_start(yfull[:, :w_], yall[0, :, :])
                nc.sync.dma_start(
                    yfull[:, w_:].rearrange("p (c f) -> p c f", c=NCORES - 1),
                    yall[1:, :, :].rearrange("c p f -> p c f"),
                )
                return yfull

            yfullA = launch_ag("A", yh["A"])
            yfullB = launch_ag("B", yh["B"])

            def lhst_ap(yfA, yfB, t):
                c, j = divmod(t, J)
                if j < JA:
                    return yfA[:, c * HA + j * B:c * HA + (j + 1) * B]
                jb = j - JA
                return yfB[:, c * HB + jb * B:c * HB + (jb + 1) * B]

            # ---- main loop ----
            for it in range(n_steps):
                last = it == n_steps - 1

                # precompute (overlaps AG + matmul on DVE)
                A_t = wk.tile([P, F], F32, tag="A", bufs=1)
                B_t = wk.tile([P, F], F32, tag="B", bufs=1)
                C_t = wk.tile([P, F], F32, tag="C", bufs=1)
                D_t = wk.tile([P, F], F32, tag="D", bufs=1)
                rE = wk.tile([P, F], F32, tag="rE", bufs=1)
                if not last:
                    V.tensor_scalar(A_t[:], un[:], 1.0 - C1, C0, MULT, ADD)
                    V.tensor_scalar(B_t[:], un[:], -A1, A1, MULT, ADD)
                    V.tensor_scalar(C_t[:], xn[:], 1.0 - C2, C2, MULT, ADD)
                    V.tensor_scalar(D_t[:], xn[:], DT, None, MULT)
                if e_v is None:
                    V.tensor_mul(rE[:], r[:], e_sb[:])

                # matmul: NG output-column groups x 64 K-tiles. Order:
                # [all groups : A-sourced K-tiles] [G0 : B-sourced]
                # -> G0 stops early; its transposes/ew/AllGather fly under
                # the remaining B-sourced matmuls of G1/G2.
                pm = [pmm.tile([B, G_BOUNDS[g + 1] - G_BOUNDS[g]], F32,
                               tag=f"mm{g}", name=f"pm{g}_{it}",
                               bufs=(2 if g < 2 else 1))
                      for g in range(NG)]
                nmm = [0] * NG

                def emit_group(g, tiles):
                    lo, hi = G_BOUNDS[g], G_BOUNDS[g + 1]
                    for t in tiles:
                        nc.tensor.matmul(
                            pm[g][:],
                            lhsT=lhst_ap(yfullA, yfullB, t),
                            rhs=w_sb[:, t * NS + lo:t * NS + hi],
                            start=(nmm[g] == 0),
                            stop=(nmm[g] == T - 1),
                        )
                        nmm[g] += 1

                def transpose_jblocks(hf, jlist):
                    """PSUM group columns -> state-layout PSUM [128, HW[hf]].

                    Per-j 128-col ACT copies so each PE transpose only waits
                    on its own small copy (~0.2us), not a whole chunk.
                    """
                    mmT_ = pT.tile([P, HW[hf]], F32, tag=f"mmT{hf}", bufs=1,
                                   name=f"mmT{hf}_{it}")
                    stage = wk.tile([B, len(jlist) * P], F32, tag=f"stage{hf}",
                                    bufs=1, name=f"stage{hf}_{it}")
                    done_g = set()
                    for k_, j in enumerate(jlist):
                        g = J_GROUP[j]
                        if g not in done_g:
                            done_g.add(g)
                            lo = max(G_BOUNDS[g], jlist[0] * P)
                            hi = min(G_BOUNDS[g + 1], (jlist[-1] + 1) * P)
                            nc.scalar.copy(
                                stage[:, lo - jlist[0] * P:hi - jlist[0] * P],
                                pm[g][:, lo - G_BOUNDS[g]:hi - G_BOUNDS[g]])
                        nc.tensor.transpose(
                            mmT_[:, k_ * B:(k_ + 1) * B],
                            stage[:, k_ * P:(k_ + 1) * P],
                            ident[:],
                        )
                    return mmT_

                for g in range(NG):
                    emit_group(g, A_TILES)
                emit_group(0, B_TILES)
                mmTA = transpose_jblocks("A", list(range(JA)))
                emit_group(1, B_TILES)
                emit_group(2, B_TILES)

                # names for per-half state pieces of this iteration
                rec_new = spool.tile([P, F], F32, tag="recfull")
                r_new = spool.tile([P, F], F32, tag="r")
                recS_new = spool.tile([P, F], F32, tag="recS")
                q = spool.tile([P, F], F32, tag="u")
                v = spool.tile([P, F], F32, tag="x")
                newy = {"A": yp.tile([P, HA], F16, tag="yA", name=f"yA_{it}"),
                        "B": yp.tile([P, HB], F16, tag="yB", name=f"yB_{it}")}

                def ew_half(hf, mmT_half):
                    sl = slice(0, HA) if hf == "A" else slice(HA, F)
                    HF = HW[hf]
                    if ds_v is not None:
                        V.scalar_tensor_tensor(rec_new[:, sl], mmT_half[:],
                                               ds_v, recS[:, sl], MULT, ADD)
                    else:
                        tmp = wk.tile([P, HF], F32, tag=f"w0{hf}", bufs=1)
                        V.tensor_mul(tmp[:], mmT_half[:], ds_sb[:, sl])
                        V.tensor_add(rec_new[:, sl], tmp[:], recS[:, sl])
                    h_ = wk.tile([P, HF], F32, tag=f"w1{hf}", bufs=1)
                    V.tensor_add(h_[:], rec_new[:, sl], ff_sb[:, sl])
                    dr_ = wk.tile([P, HF], F32, tag=f"w2{hf}", bufs=1)
                    if dt_v is not None:
                        V.tensor_scalar(dr_[:], h_[:], 0.0, dt_v, MAX, MULT)
                    else:
                        V.scalar_tensor_tensor(dr_[:], h_[:], 0.0, dt_sb[:, sl],
                                               MAX, MULT)
                    if e_v is not None:
                        V.scalar_tensor_tensor(r_new[:, sl], r[:, sl], e_v,
                                               dr_[:], MULT, ADD)
                    else:
                        V.tensor_add(r_new[:, sl], dr_[:], rE[:, sl])
                    if last:
                        return None
                    if es_v is not None:
                        V.tensor_scalar(recS_new[:, sl], rec_new[:, sl],
                                        es_v, None, MULT)
                    else:
                        V.tensor_mul(recS_new[:, sl], rec_new[:, sl], es_sb[:, sl])
                    m1_ = wk.tile([P, HF], F32, tag=f"w3{hf}", bufs=1)
                    V.tensor_mul(m1_[:], B_t[:, sl], r_new[:, sl])
                    V.tensor_add(q[:, sl], m1_[:], A_t[:, sl])
                    tt_ = wk.tile([P, HF], F32, tag=f"w4{hf}", bufs=1)
                    V.tensor_mul(tt_[:], r_new[:, sl], q[:, sl])
                    s2_ = wk.tile([P, HF], F32, tag=f"w5{hf}", bufs=1)
                    V.tensor_mul(s2_[:], D_t[:, sl], tt_[:])
                    V.scalar_tensor_tensor(v[:, sl], s2_[:], -1.0, C_t[:, sl],
                                           MULT, ADD)
                    ynew = newy[hf]
                    V.tensor_mul(ynew[:], tt_[:], v[:, sl])
                    return ynew

                yA_next = ew_half("A", mmTA)
                if not last:
                    nextA = launch_ag("A", yA_next)

                # remaining groups complete -> half B
                mmTB = transpose_jblocks("B", list(range(JA, J)))
                yB_next = ew_half("B", mmTB)
                if not last:
                    nextB = launch_ag("B", yB_next)
                    yfullA, yfullB = nextA, nextB
                    un, xn, recS = q, v, recS_new
                r = r_new

            # ---- epilogue ----
            for qi in range(4):
                nc.sync.dma_start(
                    r_out[32 * qi:32 * (qi + 1), :],
                    r[32 * qi:32 * (qi + 1), :],
                )

    nc.compile()
    return nc


# ---------------------------------------------------------------------------
# host-side data marshalling
# ---------------------------------------------------------------------------

def _shard_state(v, c):
    """[B, N] float array -> core c state tile [128, 256] (f32)."""
    vs = np.asarray(v, np.float32)[:, c * NS:(c + 1) * NS]      # [32, 1024]
    return np.ascontiguousarray(
        vs.reshape(B, J, P).transpose(2, 1, 0).reshape(P, F)
    )


def _shard_vec(v, c):
    """[N] float vector -> replicated core c tile [128, 256] (f32)."""
    vs = np.asarray(v, np.float32)[c * NS:(c + 1) * NS].reshape(J, P)  # [j, p]
    t = vs.T[:, :, None]                                        # [p, j, 1]
    return np.ascontiguousarray(np.broadcast_to(t, (P, J, B)).reshape(P, F))


def _shard_w(Wab, c):
    """Wab [N, N] -> core c weight tiles [64, 128, 1024] fp16.

    w[t, p, n] = Wab[c*1024 + n, t*128 + p]
    """
    wt = np.asarray(Wab, np.float32)[c * NS:(c + 1) * NS, :].T  # [8192, 1024]
    return np.ascontiguousarray(wt.astype(np.float16).reshape(T, P, NS))


def _unshard_out(tiles):
    """list of 8 [128, 256] tiles -> [32, 8192] f32."""
    out = np.empty((B, N), np.float32)
    for c, tl in enumerate(tiles):
        out[:, c * NS:(c + 1) * NS] = (
            np.asarray(tl, np.float32).reshape(P, J, B).transpose(2, 1, 0)
            .reshape(B, NS)
        )
    return out


def make_in_maps(rates, rec_input, ff_input, Wab, u_stp, x_stp,
                 exp_dt_tau, dt_tau, exp_dt_tau_syn, dt_tau_syn):
    recs_full = (np.asarray(exp_dt_tau_syn, np.float32)[None, :]
                 * np.asarray(rec_input, np.float32))
    in_maps = []
    for c in range(NCORES):
        in_maps.append({
            "w": _shard_w(Wab, c),
            "r0": _shard_state(rates, c),
            "recs0": _shard_state(recs_full, c),
            "u0": _shard_state(u_stp, c),
            "x0": _shard_state(x_stp, c),
            "ff": _shard_state(ff_input, c),
            "es": _shard_vec(exp_dt_tau_syn, c),
            "ds": _shard_vec(dt_tau_syn, c),
            "e": _shard_vec(exp_dt_tau, c),
            "dt": _shard_vec(dt_tau, c),
        })
    return in_maps


_PROGRAM_CACHE = {}


def _uniform_val(v):
    v = np.asarray(v, np.float32)
    return float(v.flat[0]) if np.all(v == v.flat[0]) else None


def _get_program(n_steps, uni):
    key = (n_steps, uni)
    if key not in _PROGRAM_CACHE:
        _PROGRAM_CACHE[key] = build_program(n_steps, uni=uni)
    return _PROGRAM_CACHE[key]


def run(trace=False, tmpdir=None, **inputs):
    n_steps = int(inputs.pop("n_steps"))
    uni = (_uniform_val(inputs["exp_dt_tau_syn"]),
           _uniform_val(inputs["dt_tau_syn"]),
           _uniform_val(inputs["exp_dt_tau"]),
           _uniform_val(inputs["dt_tau"]))
    nc = _get_program(n_steps, uni)
    in_maps = make_in_maps(**inputs)
    res = bass_utils.run_bass_kernel_spmd(
        nc, in_maps, core_ids=list(range(NCORES)), trace=trace, tmpdir=tmpdir
    )
    out = _unshard_out([m["r_out"] for m in res.results])
    return out, res


def kernel(**inputs):
    out, _ = run(**inputs)
    return out


# revision 30
# speedup vs baseline: 1.1107x; 1.1107x over previous
"""Trainium2 Bass kernel for the recurrent STP network (nn_Network_20109036880204).

Strategy: tensor-parallel over the output-neuron dim across 8 NeuronCores.
  - Each core owns a 1024-neuron shard: W_c = Wab[c*1024:(c+1)*1024, :]^T,
    stored fp16 resident in SBUF as 64 K-tiles [128, 1024] (128 KiB/partition).
  - All [B, N] state tensors live in SBUF in "state layout": tile [128, 256]
    with  tile[p, j*32 + b] = state[b, n = c*1024 + j*128 + p].
    (n on partitions -> fast 128-lane elementwise AND the matmul's stationary
    operand y^T [128, 32] is a contiguous free-dim slice.)
  - Per step: y = u'*x'*r (fp16) -> DRAM -> AllGather(8) -> y_full in SBUF ->
    128 matmuls (K=8192 in 64 tiles, N=1024 in 2 PSUM chunks) -> PE transpose
    of the [32, 1024] result back into state layout -> fused DVE update chain.
"""

import sys

for _p in ("/opt/trn_rl_repo", "/root/.axon_site/_ro/trn_rl_repo"):
    if _p not in sys.path:
        sys.path.append(_p)

import numpy as np

import concourse.bass as bass
import concourse.bacc as bacc
import concourse.mybir as mybir
import concourse.tile as tile
from concourse import bass_utils, masks

# problem constants
NCORES = 8
B = 32
N = 8192
NS = N // NCORES          # 1024 neurons per core
P = 128
J = NS // P               # 8 local K-tiles per core
T = N // P                # 64 K-tiles total
F = J * B                 # 256 = free size of a state tile
CHUNK = 512               # matmul moving free dim (one PSUM bank)
NCH = NS // CHUNK         # 2 chunks

DT = 0.01
USE = 0.03
TAU_FAC = 1.0
TAU_REC = 0.25
C1 = DT / TAU_FAC         # 0.01
C0 = DT * USE / TAU_FAC   # 3e-4
A1 = USE * DT             # 3e-4
C2 = DT / TAU_REC         # 0.04

F32 = mybir.dt.float32
F16 = mybir.dt.float16
MULT = mybir.AluOpType.mult
ADD = mybir.AluOpType.add
MAX = mybir.AluOpType.max


# Skewed A/B split: half A = first JA j-blocks (gathered early, small so its
# AllGather completes by matmul-end), half B = the rest.
JA = 3
JB = J - JA
HA = JA * B               # 96  = state-free width of half A
HB = JB * B               # 160 = width of half B
HW = {"A": HA, "B": HB}
A_TILES = [t for t in range(T) if t % J < JA]
B_TILES = [t for t in range(T) if t % J >= JA]
# output column groups (psum free widths; each <= 512 = one bank)
G_BOUNDS = [0, JA * P, JA * P + 4 * P, NS]      # [0, 384, 896, 1024]
NG = len(G_BOUNDS) - 1
# j-block -> group index
J_GROUP = [next(g for g in range(NG)
                if G_BOUNDS[g] <= j * P < G_BOUNDS[g + 1]) for j in range(J)]


def build_program(n_steps: int, uni=(None, None, None, None), n_dummy=8):
    """Build the SPMD Bass program (identical on all 8 cores).

    Two-half pipeline: each core's y shard is split into half A (j=0..3)
    and half B (j=4..7); each half is all-gathered separately so AG_A can
    fly while the tail of the matmul still runs, and the next step's
    matmul consumes A-sourced K-tiles first.
    """
    es_v, ds_v, e_v, dt_v = uni  # uniform values of the const vectors, or None

    nc = bacc.Bacc(
        "TRN2",
        target_bir_lowering=False,
        debug=False,
        num_devices=NCORES,
    )

    w_dram = nc.dram_tensor("w", [T, P, NS], F16, kind="ExternalInput")
    sd = {
        nm: nc.dram_tensor(nm, [P, F], F32, kind="ExternalInput")
        for nm in ["r0", "recs0", "u0", "x0", "ff", "es", "ds", "e", "dt"]
    }
    r_out = nc.dram_tensor("r_out", [P, F], F32, kind="ExternalOutput")

    with tile.TileContext(nc) as tc:
        with (
            tc.tile_pool(name="wpool", bufs=1) as wpool,
            tc.tile_pool(name="cpool", bufs=1) as cpool,
            tc.tile_pool(name="spool", bufs=2) as spool,
            tc.tile_pool(name="wk", bufs=2) as wk,
            tc.tile_pool(name="yp", bufs=2) as yp,
            tc.tile_pool(name="pmm", bufs=2, space="PSUM") as pmm,
            tc.tile_pool(name="pT", bufs=2, space="PSUM") as pT,
            tc.tile_pool(name="dp", bufs=3, space="DRAM") as dp,
        ):
            # ---- resident weights: 16 DMAs so they spread across queues ----
            w_sb = wpool.tile([P, T * NS], F16, tag="w")
            TB = 4  # K-tiles per DMA
            for i in range(T // TB):
                dst = w_sb[:, i * TB * NS:(i + 1) * TB * NS].rearrange(
                    "p (t n) -> p t n", t=TB
                )
                src = w_dram[i * TB:(i + 1) * TB, :, :].rearrange("t p n -> p t n")
                nc.sync.dma_start(dst, src)

            # ---- constants / initial state ----
            ff_sb = cpool.tile([P, F], F32, tag="ff")
            es_sb = cpool.tile([P, F], F32, tag="es")
            ds_sb = cpool.tile([P, F], F32, tag="ds")
            e_sb = cpool.tile([P, F], F32, tag="e")
            dt_sb = cpool.tile([P, F], F32, tag="dt")
            ident = cpool.tile([B, B], F32, tag="ident")
            for t_, nm in [(ff_sb, "ff"), (es_sb, "es"), (ds_sb, "ds"),
                           (e_sb, "e"), (dt_sb, "dt")]:
                nc.sync.dma_start(t_[:], sd[nm][:])
            masks.make_identity(nc, ident[:])

            r = spool.tile([P, F], F32, tag="r")
            recS = spool.tile([P, F], F32, tag="recS")
            u0_sb = wk.tile([P, F], F32, tag="u0", bufs=1)
            x0_sb = wk.tile([P, F], F32, tag="x0", bufs=1)
            for t_, nm in [(r, "r0"), (recS, "recs0"), (u0_sb, "u0"),
                           (x0_sb, "x0")]:
                nc.sync.dma_start(t_[:], sd[nm][:])

            V = nc.vector

            # ---- prologue: u1, x1, y0 from initial state ----
            s1 = wk.tile([P, F], F32, tag="t0", bufs=1)
            m = wk.tile([P, F], F32, tag="t1", bufs=1)
            s2 = wk.tile([P, F], F32, tag="t2", bufs=1)
            un = spool.tile([P, F], F32, tag="u")
            V.tensor_scalar(s1[:], u0_sb[:], 1.0 - C1, C0, MULT, ADD)
            V.tensor_mul(m[:], u0_sb[:], r[:])
            V.scalar_tensor_tensor(s2[:], r[:], A1, s1[:], MULT, ADD)
            V.scalar_tensor_tensor(un[:], m[:], -A1, s2[:], MULT, ADD)

            t2p = wk.tile([P, F], F32, tag="t3", bufs=1)
            t3p = wk.tile([P, F], F32, tag="t4", bufs=1)
            s4 = wk.tile([P, F], F32, tag="t5", bufs=1)
            xn = spool.tile([P, F], F32, tag="x")
            V.tensor_mul(t2p[:], x0_sb[:], r[:])
            V.tensor_mul(t3p[:], un[:], t2p[:])
            V.tensor_scalar(s4[:], x0_sb[:], 1.0 - C2, C2, MULT, ADD)
            V.scalar_tensor_tensor(xn[:], t3p[:], -DT, s4[:], MULT, ADD)

            w0 = wk.tile([P, F], F32, tag="t6", bufs=1)
            yh = {}
            V.tensor_mul(w0[:], un[:], xn[:])
            for hf, sl in (("A", slice(0, HA)), ("B", slice(HA, F))):
                yh[hf] = yp.tile([P, HW[hf]], F16, tag=f"y{hf}",
                                 name=f"y{hf}_pro")
                V.tensor_mul(yh[hf][:], w0[:, sl], r[:, sl])

            ag_counter = [0]

            def launch_ag(hf, ytile):
                """store y-half to DRAM, AllGather, DMA gathered tiles back."""
                k = ag_counter[0] = ag_counter[0] + 1
                w_ = HW[hf]
                ydr = dp.tile([P, w_], F16, tag=f"ydr{hf}", name=f"ydr{hf}_{k}")
                nc.sync.dma_start(ydr[:], ytile[:])
                yall = dp.tile([NCORES, P, w_], F16, tag=f"yall{hf}",
                               name=f"yall{hf}_{k}")
                nc.gpsimd.collective_compute(
                    "AllGather",
                    mybir.AluOpType.bypass,
                    replica_groups=[list(range(NCORES))],
                    ins=[ydr.opt()],
                    outs=[yall.opt()],
                )
                yfull = yp.tile([P, NCORES * w_], F16, tag=f"yfull{hf}",
                                name=f"yfull{hf}_{k}")
                # block c=0 first (tiny DMA) so the next step's first
                # matmuls ungate as early as possible
                nc.sync.dma_start(yfull[:, :w_], yall[0, :, :])
                nc.sync.dma_start(
                    yfull[:, w_:].rearrange("p (c f) -> p c f", c=NCORES - 1),
                    yall[1:, :, :].rearrange("c p f -> p c f"),
                )
                return yfull

            yfullA = launch_ag("A", yh["A"])
            yfullB = launch_ag("B", yh["B"])

            def lhst_ap(yfA, yfB, t):
                c, j = divmod(t, J)
                if j < JA:
                    return yfA[:, c * HA + j * B:c * HA + (j + 1) * B]
                jb = j - JA
                return yfB[:, c * HB + jb * B:c * HB + (jb + 1) * B]

            # ---- main loop ----
            for it in range(n_steps):
                last = it == n_steps - 1

                # precompute (overlaps AG + matmul on DVE)
                A_t = wk.tile([P, F], F32, tag="A", bufs=1)
                B_t = wk.tile([P, F], F32, tag="B", bufs=1)
                C_t = wk.tile([P, F], F32, tag="C", bufs=1)
                D_t = wk.tile([P, F], F32, tag="D", bufs=1)
                rE = wk.tile([P, F], F32, tag="rE", bufs=1)
                if not last:
                    V.tensor_scalar(A_t[:], un[:], 1.0 - C1, C0, MULT, ADD)
                    V.tensor_scalar(B_t[:], un[:], -A1, A1, MULT, ADD)
                    V.tensor_scalar(C_t[:], xn[:], 1.0 - C2, C2, MULT, ADD)
                    V.tensor_scalar(D_t[:], xn[:], DT, None, MULT)
                if e_v is None:
                    V.tensor_mul(rE[:], r[:], e_sb[:])

                # matmul: NG output-column groups x 64 K-tiles. Order:
                # [all groups : A-sourced K-tiles] [G0 : B-sourced]
                # -> G0 stops early; its transposes/ew/AllGather fly under
                # the remaining B-sourced matmuls of G1/G2.
                pm = [pmm.tile([B, G_BOUNDS[g + 1] - G_BOUNDS[g]], F32,
                               tag=f"mm{g}", name=f"pm{g}_{it}",
                               bufs=(2 if g < 2 else 1))
                      for g in range(NG)]
                nmm = [0] * NG

                def emit_group(g, tiles):
                    lo, hi = G_BOUNDS[g], G_BOUNDS[g + 1]
                    for t in tiles:
                        nc.tensor.matmul(
                            pm[g][:],
                            lhsT=lhst_ap(yfullA, yfullB, t),
                            rhs=w_sb[:, t * NS + lo:t * NS + hi],
                            start=(nmm[g] == 0),
                            stop=(nmm[g] == T - 1),
                        )
                        nmm[g] += 1

                def transpose_jblocks(hf, jlist):
                    """PSUM group columns -> state-layout PSUM [128, HW[hf]].

                    Per-j 128-col ACT copies so each PE transpose only waits
                    on its own small copy (~0.2us), not a whole chunk.
                    """
                    mmT_ = pT.tile([P, HW[hf]], F32, tag=f"mmT{hf}", bufs=1,
                                   name=f"mmT{hf}_{it}")
                    stage = wk.tile([B, len(jlist) * P], F32, tag=f"stage{hf}",
                                    bufs=1, name=f"stage{hf}_{it}")
                    done_g = set()
                    for k_, j in enumerate(jlist):
                        g = J_GROUP[j]
                        if g not in done_g:
                            done_g.add(g)
                            lo = max(G_BOUNDS[g], jlist[0] * P)
                            hi = min(G_BOUNDS[g + 1], (jlist[-1] + 1) * P)
                            nc.scalar.copy(
                                stage[:, lo - jlist[0] * P:hi - jlist[0] * P],
                                pm[g][:, lo - G_BOUNDS[g]:hi - G_BOUNDS[g]])
                        nc.tensor.transpose(
                            mmT_[:, k_ * B:(k_ + 1) * B],
                            stage[:, k_ * P:(k_ + 1) * P],
                            ident[:],
                        )
                    return mmT_

                for g in range(NG):
                    emit_group(g, A_TILES)
                emit_group(0, B_TILES)
                mmTA = transpose_jblocks("A", list(range(JA)))
                emit_group(1, B_TILES)
                emit_group(2, B_TILES)

                # names for per-half state pieces of this iteration
                rec_new = spool.tile([P, F], F32, tag="recfull")
                r_new = spool.tile([P, F], F32, tag="r")
                recS_new = spool.tile([P, F], F32, tag="recS")
                q = spool.tile([P, F], F32, tag="u")
                v = spool.tile([P, F], F32, tag="x")
                newy = {"A": yp.tile([P, HA], F16, tag="yA", name=f"yA_{it}"),
                        "B": yp.tile([P, HB], F16, tag="yB", name=f"yB_{it}")}

                def ew_half(hf, mmT_half):
                    sl = slice(0, HA) if hf == "A" else slice(HA, F)
                    HF = HW[hf]
                    if ds_v is not None:
                        V.scalar_tensor_tensor(rec_new[:, sl], mmT_half[:],
                                               ds_v, recS[:, sl], MULT, ADD)
                    else:
                        tmp = wk.tile([P, HF], F32, tag=f"w0{hf}", bufs=1)
                        V.tensor_mul(tmp[:], mmT_half[:], ds_sb[:, sl])
                        V.tensor_add(rec_new[:, sl], tmp[:], recS[:, sl])
                    h_ = wk.tile([P, HF], F32, tag=f"w1{hf}", bufs=1)
                    V.tensor_add(h_[:], rec_new[:, sl], ff_sb[:, sl])
                    dr_ = wk.tile([P, HF], F32, tag=f"w2{hf}", bufs=1)
                    if dt_v is not None:
                        V.tensor_scalar(dr_[:], h_[:], 0.0, dt_v, MAX, MULT)
                    else:
                        V.scalar_tensor_tensor(dr_[:], h_[:], 0.0, dt_sb[:, sl],
                                               MAX, MULT)
                    if e_v is not None:
                        V.scalar_tensor_tensor(r_new[:, sl], r[:, sl], e_v,
                                               dr_[:], MULT, ADD)
                    else:
                        V.tensor_add(r_new[:, sl], dr_[:], rE[:, sl])
                    if last:
                        return None
                    if es_v is not None:
                        V.tensor_scalar(recS_new[:, sl], rec_new[:, sl],
                                        es_v, None, MULT)
                    else:
                        V.tensor_mul(recS_new[:, sl], rec_new[:, sl], es_sb[:, sl])
                    m1_ = wk.tile([P, HF], F32, tag=f"w3{hf}", bufs=1)
                    V.tensor_mul(m1_[:], B_t[:, sl], r_new[:, sl])
                    V.tensor_add(q[:, sl], m1_[:], A_t[:, sl])
                    tt_ = wk.tile([P, HF], F32, tag=f"w4{hf}", bufs=1)
                    V.tensor_mul(tt_[:], r_new[:, sl], q[:, sl])
                    s2_ = wk.tile([P, HF], F32, tag=f"w5{hf}", bufs=1)
                    V.tensor_mul(s2_[:], D_t[:, sl], tt_[:])
                    V.scalar_tensor_tensor(v[:, sl], s2_[:], -1.0, C_t[:, sl],
                                           MULT, ADD)
                    ynew = newy[hf]
                    V.tensor_mul(ynew[:], tt_[:], v[:, sl])
                    return ynew

                yA_next = ew_half("A", mmTA)
                if not last:
                    nextA = launch_ag("A", yA_next)

                # remaining groups complete -> half B
                mmTB = transpose_jblocks("B", list(range(JA, J)))
                yB_next = ew_half("B", mmTB)
                if not last:
                    nextB = launch_ag("B", yB_next)
                    yfullA, yfullB = nextA, nextB
                    un, xn, recS = q, v, recS_new
                r = r_new

            # ---- epilogue ----
            for qi in range(4):
                nc.sync.dma_start(
                    r_out[32 * qi:32 * (qi + 1), :],
                    r[32 * qi:32 * (qi + 1), :],
                )

    nc.compile()
    return nc


# ---------------------------------------------------------------------------
# host-side data marshalling
# ---------------------------------------------------------------------------

def _shard_state(v, c):
    """[B, N] float array -> core c state tile [128, 256] (f32)."""
    vs = np.asarray(v, np.float32)[:, c * NS:(c + 1) * NS]      # [32, 1024]
    return np.ascontiguousarray(
        vs.reshape(B, J, P).transpose(2, 1, 0).reshape(P, F)
    )


def _shard_vec(v, c):
    """[N] float vector -> replicated core c tile [128, 256] (f32)."""
    vs = np.asarray(v, np.float32)[c * NS:(c + 1) * NS].reshape(J, P)  # [j, p]
    t = vs.T[:, :, None]                                        # [p, j, 1]
    return np.ascontiguousarray(np.broadcast_to(t, (P, J, B)).reshape(P, F))


def _shard_w(Wab, c):
    """Wab [N, N] -> core c weight tiles [64, 128, 1024] fp16.

    w[t, p, n] = Wab[c*1024 + n, t*128 + p]
    """
    wt = np.asarray(Wab, np.float32)[c * NS:(c + 1) * NS, :].T  # [8192, 1024]
    return np.ascontiguousarray(wt.astype(np.float16).reshape(T, P, NS))


def _unshard_out(tiles):
    """list of 8 [128, 256] tiles -> [32, 8192] f32."""
    out = np.empty((B, N), np.float32)
    for c, tl in enumerate(tiles):
        out[:, c * NS:(c + 1) * NS] = (
            np.asarray(tl, np.float32).reshape(P, J, B).transpose(2, 1, 0)
            .reshape(B, NS)
        )
    return out


def make_in_maps(rates, rec_input, ff_input, Wab, u_stp, x_stp,
                 exp_dt_tau, dt_tau, exp_dt_tau_syn, dt_tau_syn):
    recs_full = (np.asarray(exp_dt_tau_syn, np.float32)[None, :]
                 * np.asarray(rec_input, np.float32))
    in_maps = []
    for c in range(NCORES):
        in_maps.append({
            "w": _shard_w(Wab, c),
            "r0": _shard_state(rates, c),
            "recs0": _shard_state(recs_full, c),
            "u0": _shard_state(u_stp, c),
            "x0": _shard_state(x_stp, c),
            "ff": _shard_state(ff_input, c),
            "es": _shard_vec(exp_dt_tau_syn, c),
            "ds": _shard_vec(dt_tau_syn, c),
            "e": _shard_vec(exp_dt_tau, c),
            "dt": _shard_vec(dt_tau, c),
        })
    return in_maps


_PROGRAM_CACHE = {}


def _uniform_val(v):
    v = np.asarray(v, np.float32)
    return float(v.flat[0]) if np.all(v == v.flat[0]) else None


def _get_program(n_steps, uni):
    key = (n_steps, uni)
    if key not in _PROGRAM_CACHE:
        _PROGRAM_CACHE[key] = build_program(n_steps, uni=uni)
    return _PROGRAM_CACHE[key]


def run(trace=False, tmpdir=None, **inputs):
    n_steps = int(inputs.pop("n_steps"))
    uni = (_uniform_val(inputs["exp_dt_tau_syn"]),
           _uniform_val(inputs["dt_tau_syn"]),
           _uniform_val(inputs["exp_dt_tau"]),
           _uniform_val(inputs["dt_tau"]))
    nc = _get_program(n_steps, uni)
    in_maps = make_in_maps(**inputs)
    res = bass_utils.run_bass_kernel_spmd(
        nc, in_maps, core_ids=list(range(NCORES)), trace=trace, tmpdir=tmpdir
    )
    out = _unshard_out([m["r_out"] for m in res.results])
    return out, res


def kernel(**inputs):
    out, _ = run(**inputs)
    return out
